# revision 24
# baseline (speedup 1.0000x reference)
"""Trainium2 Bass kernel for the EquivarLayer GNN message-passing problem.

Reference computation (all fp32):
    i1g    = tanh(i1 @ W_ii + b_ii)                       # (n_pairs, R)
    ix     = p3[j_idx] * i1g[:,None,:] + d3[:,:,None] * i1g[:,None,:]
    p3_new = segment_sum(ix, i_idx, n_atoms) @ W_pp       # (n_atoms, 3, R)
    dotted = einsum('ixr,ixr->ir', p3_new, p3_new)        # (n_atoms, R)

Sharding: pairs are sorted by destination atom i and sharded by destination
atom *range* (2500 atoms per core).  Every pair whose output lands on core
c's atoms is processed on core c, so each core's segment-sum is complete
locally and no collective is needed.  The atom table p3 is replicated (each
core gathers the j-rows it needs straight from HBM).

Per core the 2500 atoms are split into 20 blocks of 125 atoms.  Each
block's pairs are padded up to `npc` chunks of 128 pairs (uniform across
blocks/cores so all 8 cores run the identical program).  Per chunk:
  - PE: i1g = i1T_chunk.T @ W_ii  (+bias via a K=1 accumulate matmul),
    tanh applied by the scalar engine in groups of 4 chunks.
  - DMA gather (batched, ~11 chunks per instruction) fetches p3[j] rows,
    one pair-row per partition.
  - DVE: one fused scalar_tensor_tensor per spatial axis x computes
    (p3[j]_x + d3_x) * i1g.
  - Pool: builds the 128x125 one-hot "selection" matrix from the local
    atom index via is_equal against an iota row.
  - PE: psum_block(125,384) += onehot.T @ ix   (scatter-add as matmul).
Per block: transpose (PE), apply W_pp (PE), square+reduce for dotted (DVE),
results DMAed out in (R, atom)-transposed layout; the host undoes the
transpose/permutation and concatenates core shards.
"""

import numpy as np

N_ATOMS = 20000
N_PAIRS = 640000
R = 128
NCORES = 8
APC = N_ATOMS // NCORES      # 2500 atoms per core
BA = 125                     # atoms per block (one-hot width, <=128)
NBLK = APC // BA             # 20 blocks per core
P = 128                      # pairs per chunk (matmul contraction)
X3 = 3 * R                   # 384 = flattened (x, r) row of p3


def _plan(ind_2):
    """Sort pairs by destination atom; assign each pair a padded slot."""
    i_idx = np.asarray(ind_2[:, 0], dtype=np.int64)
    order = np.argsort(i_idx, kind="stable")
    i_s = i_idx[order]
    gblk = i_s // BA                                   # global block id
    counts = np.bincount(gblk, minlength=NCORES * NBLK)
    npc = int(np.ceil(counts.max() / P))               # chunks per block
    S = npc * P                                        # pair slots per block
    starts = np.zeros(NCORES * NBLK + 1, np.int64)
    np.cumsum(counts, out=starts[1:])
    rank = np.arange(len(order), dtype=np.int64) - starts[gblk]
    slot = gblk * S + rank
    return order, i_s, slot, npc, S


def _make_shards(ind_2, p3, i1, d3):
    order, i_s, slot, npc, S = _plan(ind_2)
    j_s = np.asarray(ind_2[:, 1], dtype=np.int64)[order]
    tot = NCORES * NBLK * S
    nch = NBLK * npc                                   # chunks per core

    j_pad = np.zeros(tot, np.int16)
    j_pad[slot] = j_s.astype(np.int16)
    iloc = np.full(tot, -1.0, np.float32)              # -1 -> zero one-hot col
    iloc[slot] = (i_s % BA).astype(np.float32)
    i1_pad = np.zeros((tot, R), np.float32)
    i1_pad[slot] = np.asarray(i1, np.float32)[order]
    d3_pad = np.zeros((tot, 3), np.float32)
    d3_pad[slot] = np.asarray(d3, np.float32)[order]

    shards = []
    for c in range(NCORES):
        sl = slice(c * NBLK * S, (c + 1) * NBLK * S)
        # (NBLK, R, S): per block, per-partition-contiguous transposed i1
        i1t = np.ascontiguousarray(
            i1_pad[sl].reshape(NBLK, S, R).transpose(0, 2, 1))
        # gather index layout per block: idx g -> [g%16, g//16], tiled to
        # 128 rows: (NBLK, 128, S//16)
        jw16 = j_pad[sl].reshape(NBLK, S // 16, 16).transpose(0, 2, 1)
        idxw = np.ascontiguousarray(np.tile(jw16, (1, 8, 1)))
        ilocs = np.ascontiguousarray(iloc[sl].reshape(nch, P).T)
        d3c = np.ascontiguousarray(
            d3_pad[sl].reshape(nch, P, 3).transpose(1, 0, 2)).reshape(
                P, nch * 3)
        shards.append(dict(i1t=i1t, idxw=idxw, ilocs=ilocs, d3c=d3c))
    return shards, npc


def _split_sizes(n, maxsz):
    k = -(-n // maxsz)
    base, rem = divmod(n, k)
    return [base + (i < rem) for i in range(k)]


def _build(npc):
    """Build the (single, SPMD) Bass program for one core."""
    import os
    from contextlib import ExitStack

    dbg_nblk = int(os.environ.get("K_NBLK", NBLK))
    dbg_skip = set(os.environ.get("K_SKIP", "").split(","))

    import concourse.tile as tile
    from concourse import bacc, mybir

    f32 = mybir.dt.float32
    nch = NBLK * npc
    S = npc * P

    # Bacc (not plain Bass): its compile() encodes the Ant ISA instructions
    # (DMAGatherAnt) and auto-inserts the Q7 library reloads they need.
    nc = bacc.Bacc()
    p3f = nc.declare_dram_parameter("p3f", [N_ATOMS, X3], f32, isOutput=False)
    i1t = nc.declare_dram_parameter("i1t", [NBLK, R, S], f32, isOutput=False)
    idxw = nc.declare_dram_parameter(
        "idxw", [NBLK, P, S // 16], mybir.dt.int16, isOutput=False)
    ilocs = nc.declare_dram_parameter("ilocs", [P, nch], f32, isOutput=False)
    d3c = nc.declare_dram_parameter("d3c", [P, nch * 3], f32, isOutput=False)
    wii = nc.declare_dram_parameter("wii", [R, R], f32, isOutput=False)
    wpp = nc.declare_dram_parameter("wpp", [R, R], f32, isOutput=False)
    b4 = nc.declare_dram_parameter("b4", [1, 4 * R], f32, isOutput=False)
    ones = nc.declare_dram_parameter("ones", [1, R], f32, isOutput=False)
    ident = nc.declare_dram_parameter("ident", [R, R], f32, isOutput=False)
    iota = nc.declare_dram_parameter("iota", [P, BA], f32, isOutput=False)
    outp = nc.declare_dram_parameter(
        "outp", [P, NBLK * 3 * BA], f32, isOutput=True)
    outd = nc.declare_dram_parameter("outd", [P, NBLK * BA], f32, isOutput=True)

    gather_sizes = _split_sizes(npc, 11)   # chunks per gather instruction

    with tile.TileContext(nc) as tc, ExitStack() as ctx:
        # dma_gather consumes a fresh Pool register per call for
        # num_idxs_reg (only ~48 exist) — allocate one per distinct size.
        nidx_regs = {gsz: nc.gpsimd.to_reg(gsz * P)
                     for gsz in sorted(set(gather_sizes))}
        cpool = ctx.enter_context(tc.tile_pool(name="consts", bufs=1))
        i1pool = ctx.enter_context(tc.tile_pool(name="i1t", bufs=2))
        ipool = ctx.enter_context(tc.tile_pool(name="idx", bufs=2))
        gpool = ctx.enter_context(
            tc.tile_pool(name="gather", bufs=len(gather_sizes) + 1))
        igpool = ctx.enter_context(tc.tile_pool(name="i1g", bufs=2))
        wpool = ctx.enter_context(tc.tile_pool(name="work", bufs=3))
        tpool = ctx.enter_context(tc.tile_pool(name="tail", bufs=2))
        pp_out = ctx.enter_context(tc.tile_pool(name="ps_out", bufs=2, space="PSUM"))
        pp_ig = ctx.enter_context(tc.tile_pool(name="ps_ig", bufs=2, space="PSUM"))
        pp_t = ctx.enter_context(tc.tile_pool(name="ps_t", bufs=2, space="PSUM"))
        pp_f = ctx.enter_context(tc.tile_pool(name="ps_f", bufs=2, space="PSUM"))

        # ---- constants / metadata preload ----
        wii_sb = cpool.tile([R, R], f32)
        nc.sync.dma_start(out=wii_sb[:], in_=wii[:])
        wpp_sb = cpool.tile([R, R], f32)
        nc.sync.dma_start(out=wpp_sb[:], in_=wpp[:])
        b4_sb = cpool.tile([1, 4 * R], f32)
        nc.sync.dma_start(out=b4_sb[:], in_=b4[:])
        ones_sb = cpool.tile([1, R], f32)
        nc.sync.dma_start(out=ones_sb[:], in_=ones[:])
        ident_sb = cpool.tile([R, R], f32)
        nc.sync.dma_start(out=ident_sb[:], in_=ident[:])
        iota_sb = cpool.tile([P, BA], f32)
        nc.sync.dma_start(out=iota_sb[:], in_=iota[:])
        ilocs_sb = cpool.tile([P, nch], f32)
        nc.sync.dma_start(out=ilocs_sb[:], in_=ilocs[:])
        d3c_sb = cpool.tile([P, nch * 3], f32)
        nc.sync.dma_start(out=d3c_sb[:], in_=d3c[:])

        for b in range(dbg_nblk):
            i1t_sb = i1pool.tile([R, S], f32, tag="i1t")
            nc.sync.dma_start(out=i1t_sb[:], in_=i1t[b])
            idx_sb = ipool.tile([P, S // 16], mybir.dt.int16, tag="idx")
            nc.sync.dma_start(out=idx_sb[:], in_=idxw[b])

            # gathered p3[j] rows for this block, in gather-instruction groups
            gbufs = []
            goff = 0
            for gsz in gather_sizes:
                gb = gpool.tile([P, gsz, X3], f32, tag="gb")
                pos0 = goff * P
                if "gather" not in dbg_skip:
                    nc.gpsimd.dma_gather(
                        out_ap=gb[:],
                        in_ap=p3f[:],
                        idxs_ap=idx_sb[:, pos0 // 16:(pos0 + gsz * P) // 16],
                        num_idxs=gsz * P,
                        num_idxs_reg=nidx_regs[gsz],
                        elem_size=X3,
                        single_packet=False,
                    )
                gbufs.append((goff, gsz, gb))
                goff += gsz

            # i1g for the whole block, in groups of 4 chunks (one PSUM bank)
            ig_sb = igpool.tile([P, npc * R], f32, tag="i1g")
            for g0 in range(0, npc, 4):
                ng = min(4, npc - g0)
                ps_ig = pp_ig.tile([P, 4 * R], f32, tag="ig")
                skip_bias = "bias" in dbg_skip
                for k in range(ng):
                    nc.tensor.matmul(
                        out=ps_ig[:, k * R:(k + 1) * R],
                        lhsT=i1t_sb[:, (g0 + k) * P:(g0 + k + 1) * P],
                        rhs=wii_sb[:],
                        start=(k == 0), stop=(skip_bias and k == ng - 1),
                    )
                if not skip_bias:
                    nc.tensor.matmul(
                        out=ps_ig[:, :ng * R],
                        lhsT=ones_sb[:],
                        rhs=b4_sb[:, :ng * R],
                        start=False, stop=True,
                    )
                nc.scalar.activation(
                    out=ig_sb[:, g0 * R:(g0 + ng) * R], in_=ps_ig[:, :ng * R],
                    func=mybir.ActivationFunctionType.Tanh)

            # scatter-accumulate the whole block into one PSUM bank
            ps_out = pp_out.tile([BA, X3], f32, tag="po")
            gi = 0
            for c in range(npc):
                goff, gsz, gb = gbufs[gi]
                if c - goff >= gsz:
                    gi += 1
                    goff, gsz, gb = gbufs[gi]
                gslice = gb[:, c - goff, :]
                ch = b * npc + c
                ig = ig_sb[:, c * R:(c + 1) * R]

                ix = wpool.tile([P, X3], f32, tag="ix")
                if "stt" not in dbg_skip:
                    for x in range(3):
                        nc.vector.scalar_tensor_tensor(
                            out=ix[:, x * R:(x + 1) * R],
                            in0=gslice[:, x * R:(x + 1) * R],
                            scalar=d3c_sb[:, ch * 3 + x:ch * 3 + x + 1],
                            in1=ig,
                            op0=mybir.AluOpType.add,
                            op1=mybir.AluOpType.mult,
                        )

                oh = wpool.tile([P, BA], f32, tag="oh")
                if "onehot" not in dbg_skip:
                    nc.gpsimd.tensor_scalar(
                        oh[:], iota_sb[:], ilocs_sb[:, ch:ch + 1], None,
                        mybir.AluOpType.is_equal)

                if "scatter" not in dbg_skip:
                    nc.tensor.matmul(
                        out=ps_out[:], lhsT=oh[:], rhs=ix[:],
                        start=(c == 0), stop=(c == npc - 1),
                        skip_group_check=True,
                    )

            # ---- block tail: W_pp, dotted, writeback ----
            if "tail" in dbg_skip or "scatter" in dbg_skip:
                continue
            pn_sb = tpool.tile([BA, X3], f32, tag="pn")
            nc.vector.tensor_copy(pn_sb[:], ps_out[:])
            pnT_sb = tpool.tile([R, 3 * BA], f32, tag="pnT")
            for x in range(3):
                ps_t = pp_t.tile([R, BA], f32, tag="pt")
                nc.tensor.transpose(
                    out=ps_t[:], in_=pn_sb[:, x * R:(x + 1) * R],
                    identity=ident_sb[:BA, :BA])
                nc.any.tensor_copy(
                    out=pnT_sb[:, x * BA:(x + 1) * BA], in_=ps_t[:])
            ps_f = pp_f.tile([R, 3 * BA], f32, tag="pf")
            nc.tensor.matmul(out=ps_f[:], lhsT=wpp_sb[:], rhs=pnT_sb[:],
                             start=True, stop=True)
            o_sb = tpool.tile([R, 3 * BA], f32, tag="osb")
            nc.any.tensor_copy(out=o_sb[:], in_=ps_f[:])
            nc.sync.dma_start(
                out=outp[:, b * 3 * BA:(b + 1) * 3 * BA], in_=o_sb[:])

            sq_sb = tpool.tile([R, 3 * BA], f32, tag="sq")
            nc.scalar.square(out=sq_sb[:], in_=ps_f[:])
            dot_sb = tpool.tile([R, BA], f32, tag="dot")
            nc.vector.tensor_tensor(
                out=dot_sb[:], in0=sq_sb[:, :BA], in1=sq_sb[:, BA:2 * BA],
                op=mybir.AluOpType.add)
            nc.vector.tensor_tensor(
                out=dot_sb[:], in0=dot_sb[:], in1=sq_sb[:, 2 * BA:3 * BA],
                op=mybir.AluOpType.add)
            nc.sync.dma_start(out=outd[:, b * BA:(b + 1) * BA], in_=dot_sb[:])

    nc.compile()
    return nc


def _in_maps(inputs):
    shards, npc = _make_shards(
        inputs["ind_2"], inputs["p3"], inputs["i1"], inputs["d3"])
    p3f = np.ascontiguousarray(
        np.asarray(inputs["p3"], np.float32).reshape(N_ATOMS, X3))
    wii = np.ascontiguousarray(np.asarray(inputs["W_ii"], np.float32))
    wpp = np.ascontiguousarray(np.asarray(inputs["W_pp"], np.float32))
    b_ii = np.asarray(inputs["b_ii"], np.float32)
    b4 = np.ascontiguousarray(np.tile(b_ii.reshape(1, R), (1, 4)))
    onesr = np.ones((1, R), np.float32)
    ident = np.eye(R, dtype=np.float32)
    iota = np.ascontiguousarray(
        np.tile(np.arange(BA, dtype=np.float32), (P, 1)))
    in_maps = []
    for c in range(NCORES):
        m = dict(shards[c])
        m.update(p3f=p3f, wii=wii, wpp=wpp, b4=b4, ones=onesr,
                 ident=ident, iota=iota)
        in_maps.append(m)
    return in_maps, npc


def _unshard(results):
    p3_new = np.empty((N_ATOMS, 3, R), np.float32)
    dotted = np.empty((N_ATOMS, R), np.float32)
    for c, res in enumerate(results):
        op = res["outp"].reshape(R, NBLK, 3, BA).transpose(1, 3, 2, 0)
        p3_new[c * APC:(c + 1) * APC] = op.reshape(APC, 3, R)
        od = res["outd"].reshape(R, NBLK, BA).transpose(1, 2, 0)
        dotted[c * APC:(c + 1) * APC] = od.reshape(APC, R)
    return p3_new, dotted


def kernel(ind_2, p3, i1, d3, W_ii, b_ii, W_pp, _trace=False, _tmpdir=None):
    from concourse.bass_utils import run_bass_kernel_spmd

    inputs = dict(ind_2=ind_2, p3=p3, i1=i1, d3=d3,
                  W_ii=W_ii, b_ii=b_ii, W_pp=W_pp)
    in_maps, npc = _in_maps(inputs)
    nc = _build(npc)
    res = run_bass_kernel_spmd(
        nc, in_maps, list(range(NCORES)), trace=_trace, tmpdir=_tmpdir)
    out = _unshard(res.results)
    if _trace:
        return out, res
    return out
